# revision 1
# baseline (speedup 1.0000x reference)
"""3x3 neighborhood cosine-similarity sum (minus self) on 8 TRN2 NeuronCores.

Input:  input_image [1024, 1024, 1, 128] float32  (H, W, 1, C)
Output: sim [1024, 1024] float32

Algorithm per pixel: sim = <xn, BoxSum3x3(xn)> - 1, where xn = x / max(||x||, eps).

Sharding: H rows split 128/core across 8 cores; each core gets its 128 rows
plus 1 halo row above/below (zero rows at the image edges), i.e. [130, 1024, 128].

Per-core layout ("layout A"): SBUF tiles [128 part = w%128, free = (j=w//128, c)].
 - ss      : per-chunk fused tensor_tensor_reduce (x*x, add-accum) on DVE
 - inv     : sqrt(ss + 1e-16) on ACT, reciprocal on DVE
 - xn      : per-chunk tensor_scalar mult (f32 in, bf16 out) on DVE
 - vertical: 2 tensor_tensor adds (bf16) on DVE over the xn row ring
 - horizontal: band-matrix matmuls on PE (tridiag within chunk + 2 cross-chunk
   single-entry matrices), accumulated in PSUM
 - evac    : PSUM -> SBUF bf16 copy on ACT
 - dot     : per-chunk fused tensor_tensor_reduce (xn*S, add-accum, init=-1) on DVE
Output rows staged as [blk, p, j*16+rr] and untangled on the host.
"""

import numpy as np
import ml_dtypes

import sys

for _p in ("/opt/trn_rl_repo",):
    if _p not in sys.path:
        sys.path.insert(0, _p)

import concourse.bass as bass
import concourse.bacc as bacc
import concourse.mybir as mybir
import concourse.tile as tile
from concourse.bass_utils import run_bass_kernel_spmd

F32 = mybir.dt.float32
BF16 = mybir.dt.bfloat16
ALU = mybir.AluOpType
ACTF = mybir.ActivationFunctionType

H, W, C = 1024, 1024, 128
NCORES = 8
ROWS_PER_CORE = H // NCORES          # 128
NJ = W // 128                        # 8 w-chunks
RBLK = 16                            # output rows per staging block


def build_consts():
    """Host-side constant matrices for the horizontal box-sum matmuls."""
    t = np.zeros((128, 128), np.float32)
    for k in range(128):
        for m in (k - 1, k, k + 1):
            if 0 <= m < 128:
                t[k, m] = 1.0
    el = np.zeros((128, 128), np.float32)
    el[127, 0] = 1.0
    er = np.zeros((128, 128), np.float32)
    er[0, 127] = 1.0
    to_bf = lambda a: a.astype(ml_dtypes.bfloat16)
    return to_bf(t), to_bf(el), to_bf(er)


def build_bass(n_out_rows=ROWS_PER_CORE):
    """Build the per-core Bass graph. n_out_rows output rows need
    n_out_rows + 2 input rows (zero-padded halo included by the host)."""
    n_in = n_out_rows + 2
    nblk = (n_out_rows + RBLK - 1) // RBLK

    nc = bacc.Bacc(None, target_bir_lowering=False)
    x_dram = nc.declare_dram_parameter("x", [n_in, W, C], F32, isOutput=False)
    band_dram = nc.declare_dram_parameter("band", [128, 128], BF16, isOutput=False)
    el_dram = nc.declare_dram_parameter("el", [128, 128], BF16, isOutput=False)
    er_dram = nc.declare_dram_parameter("er", [128, 128], BF16, isOutput=False)
    out_dram = nc.declare_dram_parameter(
        "out", [nblk, 128, NJ * RBLK], F32, isOutput=True
    )

    with tile.TileContext(nc) as tc:
        with (
            tc.tile_pool(name="consts", bufs=1) as cpool,
            tc.tile_pool(name="xin", bufs=3) as xpool,
            tc.tile_pool(name="sq", bufs=2) as sqpool,
            tc.tile_pool(name="norm", bufs=3) as npool,
            tc.tile_pool(name="xn", bufs=5) as xnpool,
            tc.tile_pool(name="v", bufs=2) as vpool,
            tc.tile_pool(name="sb", bufs=2) as sbpool,
            tc.tile_pool(name="pd", bufs=2) as pdpool,
            tc.tile_pool(name="sim", bufs=2) as simpool,
            tc.tile_pool(name="psum", bufs=2, space="PSUM") as psumpool,
        ):
            band = cpool.tile([128, 128], BF16, tag="band")
            el = cpool.tile([128, 128], BF16, tag="el")
            er = cpool.tile([128, 128], BF16, tag="er")
            nc.sync.dma_start(band[:], band_dram[:])
            nc.sync.dma_start(el[:], el_dram[:])
            nc.sync.dma_start(er[:], er_dram[:])
            eps_bias = cpool.tile([128, 1], F32, tag="eps")
            nc.gpsimd.memset(eps_bias[:], 1e-16)

            xn_rows = [None] * n_in
            simt = None

            for h in range(n_in):
                # ---- load row h: [128 p, NJ j, 128 c], w = j*128 + p
                xt = xpool.tile([128, NJ, C], F32, tag="xt")
                nc.sync.dma_start(
                    xt[:], x_dram[h].rearrange("(j p) c -> p j c", p=128)
                )

                # ---- ss[p, j] = sum_c x^2 (fused mult+reduce per chunk)
                sq = sqpool.tile([128, NJ, C], BF16, tag="sq")
                ssr = npool.tile([128, NJ], F32, tag="ssr")
                import os
                if os.environ.get("SS_POW", "0") == "1":
                    for j in range(NJ):
                        nc.vector.tensor_scalar(
                            sq[:, j, :],
                            xt[:, j, :],
                            2.0,
                            0.0,
                            ALU.pow,
                            ALU.add,
                            accum_out=ssr[:, j : j + 1],
                        )
                else:
                    for j in range(NJ):
                        nc.vector.scalar_tensor_tensor(
                            sq[:, j, :],
                            xt[:, j, :],
                            1.0,
                            xt[:, j, :],
                            ALU.mult,
                            ALU.mult,
                            accum_out=ssr[:, j : j + 1],
                        )

                # ---- inv = 1 / sqrt(ss + 1e-16)   (1e-16 keeps zero rows finite,
                #      matches reference x / max(||x||, 1e-8) exactly for zeros)
                snorm = npool.tile([128, NJ], F32, tag="snorm")
                nc.scalar.activation(snorm[:], ssr[:], ACTF.Sqrt, bias=eps_bias[:])
                sinv = npool.tile([128, NJ], F32, tag="sinv")
                nc.vector.reciprocal(sinv[:], snorm[:])

                # ---- xn = x * inv  (f32 -> bf16), per chunk (per-partition scalar)
                xnt = xnpool.tile([128, NJ, C], BF16, tag="xnt")
                for j in range(NJ):
                    nc.vector.tensor_scalar(
                        xnt[:, j, :],
                        xt[:, j, :],
                        sinv[:, j : j + 1],
                        None,
                        ALU.mult,
                    )
                xn_rows[h] = xnt

                if h < 2:
                    continue

                # ---- output row r (padded coords); local output index r-1
                r = h - 1
                ro = r - 1  # 0..n_out_rows-1
                xa, xb_, xc = xn_rows[r - 1], xn_rows[r], xn_rows[r + 1]
                xn_rows[r - 1] = None

                vtmp = vpool.tile([128, NJ, C], BF16, tag="vtmp")
                nc.vector.tensor_add(vtmp[:], xa[:], xc[:])
                vt = vpool.tile([128, NJ, C], BF16, tag="vt")
                nc.vector.tensor_add(vt[:], vtmp[:], xb_[:])

                # ---- horizontal box sum on PE: S = T@V + EL@V(j-1) + ER@V(j+1)
                S = psumpool.tile([128, NJ, C], F32, tag="S")
                hj = NJ // 2  # PSUM bank boundary at j=4 (512 f32)
                nc.tensor.matmul(
                    S[:, 0:hj, :], band[:], vt[:, 0:hj, :], start=True, stop=False
                )
                nc.tensor.matmul(
                    S[:, hj:NJ, :], band[:], vt[:, hj:NJ, :], start=True, stop=False
                )
                nc.tensor.matmul(
                    S[:, 1:hj, :], el[:], vt[:, 0 : hj - 1, :], start=False, stop=False
                )
                nc.tensor.matmul(
                    S[:, hj:NJ, :], el[:], vt[:, hj - 1 : NJ - 1, :],
                    start=False, stop=False,
                )
                nc.tensor.matmul(
                    S[:, 0:hj, :], er[:], vt[:, 1 : hj + 1, :], start=False, stop=True
                )
                nc.tensor.matmul(
                    S[:, hj : NJ - 1, :], er[:], vt[:, hj + 1 : NJ, :],
                    start=False, stop=True,
                )

                # ---- evacuate S to SBUF as bf16 (ACT)
                sb = sbpool.tile([128, NJ, C], BF16, tag="sbt")
                nc.scalar.activation(sb[:], S[:], ACTF.Copy)

                # ---- sim[p, j] = sum_c xn*S - 1 (fused, init = -1)
                if ro % RBLK == 0:
                    simt = simpool.tile([128, NJ * RBLK], F32, tag="simt")
                rr = ro % RBLK
                pd = pdpool.tile([128, NJ, C], BF16, tag="pd")
                for j in range(NJ):
                    col = j * RBLK + rr
                    nc.vector.scalar_tensor_tensor(
                        pd[:, j, :],
                        xb_[:, j, :],
                        1.0,
                        sb[:, j, :],
                        ALU.mult,
                        ALU.mult,
                        accum_out=simt[:, col : col + 1],
                    )

                if ro % RBLK == RBLK - 1 or ro == n_out_rows - 1:
                    blk = ro // RBLK
                    simo = simpool.tile([128, NJ * RBLK], F32, tag="simo")
                    nc.vector.tensor_scalar(
                        simo[:], simt[:], -1.0, None, ALU.add
                    )
                    nc.sync.dma_start(out_dram[blk], simo[:])

    nc.compile()
    return nc


def shard_inputs(input_image):
    """input_image [H, W, 1, C] f32 -> per-core in_maps."""
    x = np.asarray(input_image).reshape(H, W, C).astype(np.float32, copy=False)
    xp = np.zeros((H + 2, W, C), np.float32)
    xp[1 : H + 1] = x
    band, el, er = build_consts()
    in_maps = []
    for core in range(NCORES):
        lo = core * ROWS_PER_CORE
        shard = np.ascontiguousarray(xp[lo : lo + ROWS_PER_CORE + 2])
        in_maps.append({"x": shard, "band": band, "el": el, "er": er})
    return in_maps


def unshard_output(results):
    """results[i]['out'] [nblk, 128, NJ*RBLK] -> [H, W] f32."""
    out = np.empty((H, W), np.float32)
    for core in range(NCORES):
        st = np.asarray(results[core]["out"])  # [nblk, 128, NJ*RBLK]
        nblk = st.shape[0]
        st = st.reshape(nblk, 128, NJ, RBLK)  # [blk, p, j, rr]
        sim = st.transpose(0, 3, 2, 1).reshape(nblk * RBLK, W)  # [h_local, w]
        out[core * ROWS_PER_CORE : (core + 1) * ROWS_PER_CORE] = sim[:ROWS_PER_CORE]
    return out


_NC_CACHE = {}


def get_nc():
    if "nc" not in _NC_CACHE:
        _NC_CACHE["nc"] = build_bass()
    return _NC_CACHE["nc"]


def kernel(input_image):
    nc = get_nc()
    in_maps = shard_inputs(input_image)
    res = run_bass_kernel_spmd(nc, in_maps, list(range(NCORES)))
    return unshard_output(res.results)


if __name__ == "__main__":
    rng = np.random.default_rng(0)
    x = rng.standard_normal((H, W, 1, C), dtype=np.float32)
    out = kernel(x)
    print(out.shape, out.dtype, out[:2, :4])



# revision 12
# speedup vs baseline: 1.3647x; 1.3647x over previous
"""3x3 neighborhood cosine-similarity sum (minus self) on 8 TRN2 NeuronCores.

Input:  input_image [1024, 1024, 1, 128] float32  (H, W, 1, C)
Output: sim [1024, 1024] float32

Algorithm per pixel: sim = <xn, BoxSum3x3(xn)> - 1, xn = x / max(||x||, eps).

Sharding: H rows split 128/core across 8 cores, 1-row halo (zeros at edges).

V3 design ("overlap chunks + fused vertical"):
 - w split into 9 OVERLAPPING chunks of 128 (stride 126): chunk j holds
   padded w-range [off_j, off_j+128), off = [0,126,...,882, 898] over a
   zero-padded width of 1026. Valid outputs at partitions p in [1,126].
   This makes the horizontal box sum a single in-chunk tridiagonal band
   matmul (no cross-chunk fixup matmuls).
 - vertical 3-row sum fused into PE: S_r = sum_{h=r-1..r+1} T @ xn_h
   accumulated in PSUM across three row iterations (start at h=r-1,
   stop at h=r+1). No DVE vertical adds at all.
 - ss (sum of squares) per chunk via tensor_scalar pow+accum on DVE,
   with some chunks offloaded to ACT (activation Square + accum_out).
 - xn = x * rsqrt(ss+eps) per chunk: split DVE tensor_scalar / ACT copy
   with scale AP.
 - evac: one ACT activation Copy per output row PSUM->SBUF bf16.
 - dot: 9x scalar_tensor_tensor bf16 with accum into staging columns.

Host: pre-chunks the shard to [130, 128, 9, 128] f32 so each partition's
DMA line is 4608B contiguous; output staged [blk, p, j*16+rr], untangled
on host.
"""

import numpy as np
import ml_dtypes

import sys

for _p in ("/opt/trn_rl_repo",):
    if _p not in sys.path:
        sys.path.insert(0, _p)

import concourse.bass as bass
import concourse.bacc as bacc
import concourse.mybir as mybir
import concourse.tile as tile
from concourse.bass_utils import run_bass_kernel_spmd

F32 = mybir.dt.float32
BF16 = mybir.dt.bfloat16
ALU = mybir.AluOpType
ACTF = mybir.ActivationFunctionType

H, W, C = 1024, 1024, 128
NCORES = 8
ROWS_PER_CORE = H // NCORES          # 128
NJ = 9                               # overlapping w-chunks
WPAD = W + 2                         # zero col at 0 and 1025
CHUNK_OFF = [126 * j for j in range(8)] + [WPAD - 128]  # [0..882, 898]
RBLK = 16                            # output rows per staging block

# engine split for per-chunk work (tuned from trace):
SS_ACT = (0, 2, 4, 6)          # chunk js whose ss runs on ACT
XN_ACT = (1, 5)                # chunk js whose xn runs on ACT
SS_POW = False                 # pow not ISA-valid for tensor_scalar+reduce


def build_t_band():
    t = np.zeros((128, 128), np.float32)
    for k in range(128):
        for m in (k - 1, k, k + 1):
            if 0 <= m < 128:
                t[k, m] = 1.0
    return t.astype(ml_dtypes.bfloat16)


def build_bass(n_out_rows=ROWS_PER_CORE):
    n_in = n_out_rows + 2
    nblk = (n_out_rows + RBLK - 1) // RBLK

    nc = bacc.Bacc(None, target_bir_lowering=False)
    x_dram = nc.declare_dram_parameter("x", [n_in, 128, NJ, C], F32, isOutput=False)
    band_dram = nc.declare_dram_parameter("band", [128, 128], BF16, isOutput=False)
    out_dram = nc.declare_dram_parameter(
        "out", [nblk, 128, NJ * RBLK], F32, isOutput=True
    )

    with tile.TileContext(nc) as tc:
        with (
            tc.tile_pool(name="consts", bufs=1) as cpool,
            tc.tile_pool(name="xin", bufs=4) as xpool,
            tc.tile_pool(name="dmy", bufs=2) as dpool,
            tc.tile_pool(name="norm", bufs=4) as npool,
            tc.tile_pool(name="xn", bufs=6) as xnpool,
            tc.tile_pool(name="sb", bufs=3) as sbpool,
            tc.tile_pool(name="sim", bufs=2) as simpool,
            tc.tile_pool(name="psum", bufs=1, space="PSUM") as psumpool,
        ):
            band = cpool.tile([128, 128], BF16, tag="band")
            nc.sync.dma_start(band[:], band_dram[:])
            eps_bias = cpool.tile([128, 1], F32, tag="eps")
            nc.gpsimd.memset(eps_bias[:], 1e-16)

            # PSUM: accumulation groups claim whole 2KB banks, so slots must be
            # bank-exclusive. A-slots (chunks 0..7) = 2 banks x3 rotating;
            # B-slots (chunk 8) = 1 full bank x2 rotating. 6 + 2 = 8 banks.
            psum_all = psumpool.tile([128, 32, C], F32, tag="S3")

            xn_rows = [None] * n_in
            simt = None

            for h in range(n_in):
                # ---- load row h: [128 p, 9 j, 128 c]
                xt = xpool.tile([128, NJ, C], F32, tag="xt")
                nc.sync.dma_start(xt[:], x_dram[h])

                # ---- ss[p, j] = sum_c x^2 per chunk (DVE / ACT split)
                ssr = npool.tile([128, NJ], F32, tag="ssr")
                sqd = dpool.tile([128, C], BF16, tag="sqd")
                if SS_ACT:
                    sqa = dpool.tile([128, C], BF16, tag="sqa")
                else:
                    sqa = None
                for j in range(NJ):
                    if j in SS_ACT:
                        nc.scalar.activation(
                            sqa[:], xt[:, j, :], ACTF.Square,
                            accum_out=ssr[:, j : j + 1],
                        )
                    elif SS_POW:
                        nc.vector.tensor_scalar(
                            sqd[:], xt[:, j, :], 2.0, 0.0, ALU.pow, ALU.add,
                            accum_out=ssr[:, j : j + 1],
                        )
                    else:
                        nc.vector.scalar_tensor_tensor(
                            sqd[:], xt[:, j, :], 1.0, xt[:, j, :],
                            ALU.mult, ALU.mult,
                            accum_out=ssr[:, j : j + 1],
                        )

                # ---- inv = 1 / sqrt(ss + 1e-16)
                snorm = npool.tile([128, NJ], F32, tag="snorm")
                nc.scalar.activation(snorm[:], ssr[:], ACTF.Sqrt, bias=eps_bias[:])
                sinv = npool.tile([128, NJ], F32, tag="sinv")
                nc.vector.reciprocal(sinv[:], snorm[:])

                # ---- xn = x * inv  (f32 -> bf16), per chunk (DVE / ACT split)
                xnt = xnpool.tile([128, NJ, C], BF16, tag="xnt")
                for j in range(NJ):
                    if j in XN_ACT:
                        nc.scalar.activation(
                            xnt[:, j, :], xt[:, j, :], ACTF.Copy,
                            scale=sinv[:, j : j + 1],
                        )
                    else:
                        nc.vector.tensor_scalar(
                            xnt[:, j, :], xt[:, j, :], sinv[:, j : j + 1],
                            None, ALU.mult,
                        )
                xn_rows[h] = xnt

                # ---- PE: accumulate T @ xn_h into S_{h-1}, S_h, S_{h+1}.
                # Emission order matters: stop-pass for S_{h-1}, then its
                # evacs (so the B-bank WAR orders start-after-evac), then
                # mid/start passes.
                def band_pass(r, start, stop):
                    ka_ = (r % 3) * 8          # A-slot base chunk (2 banks)
                    kb_ = 24 + (r % 2) * 4     # B-slot base chunk (1 bank)
                    nc.tensor.matmul(
                        psum_all[:, ka_ : ka_ + 4, :], band[:],
                        xnt[:, 0:4, :], start=start, stop=stop,
                    )
                    nc.tensor.matmul(
                        psum_all[:, ka_ + 4 : ka_ + 8, :], band[:],
                        xnt[:, 4:8, :], start=start, stop=stop,
                    )
                    nc.tensor.matmul(
                        psum_all[:, kb_ : kb_ + 1, :], band[:],
                        xnt[:, 8:9, :], start=start, stop=stop,
                    )

                if h >= 2:
                    band_pass(h - 1, start=False, stop=True)

                    # ---- row r = h-1 complete: evac (B first: frees its bank)
                    r = h - 1
                    ro = r - 1  # 0..n_out_rows-1
                    ka = (r % 3) * 8
                    kb = 24 + (r % 2) * 4
                    xb_ = xn_rows[r]
                    xn_rows[r - 1] = None
                    sb = sbpool.tile([128, NJ, C], BF16, tag="sbt")
                    nc.scalar.activation(
                        sb[:, 8:9, :], psum_all[:, kb : kb + 1, :], ACTF.Copy
                    )
                    nc.scalar.activation(
                        sb[:, 0:8, :], psum_all[:, ka : ka + 8, :], ACTF.Copy
                    )

                if h <= n_out_rows - 1:
                    band_pass(h + 1, start=True, stop=False)
                if 1 <= h <= n_out_rows:
                    band_pass(h, start=False, stop=False)

                if h < 2:
                    continue

                if ro % RBLK == 0:
                    simt = simpool.tile([128, NJ * RBLK], F32, tag="simt")
                rr = ro % RBLK
                pd = dpool.tile([128, C], BF16, tag="pd")
                for j in range(NJ):
                    col = j * RBLK + rr
                    nc.vector.scalar_tensor_tensor(
                        pd[:],
                        xb_[:, j, :],
                        1.0,
                        sb[:, j, :],
                        ALU.mult,
                        ALU.mult,
                        accum_out=simt[:, col : col + 1],
                    )

                if ro % RBLK == RBLK - 1 or ro == n_out_rows - 1:
                    blk = ro // RBLK
                    simo = simpool.tile([128, NJ * RBLK], F32, tag="simo")
                    nc.vector.tensor_scalar(
                        simo[:], simt[:], -1.0, None, ALU.add
                    )
                    nc.sync.dma_start(out_dram[blk], simo[:])

    nc.compile()
    return nc


def shard_inputs(input_image):
    """input_image [H, W, 1, C] f32 -> per-core in_maps (pre-chunked)."""
    x = np.asarray(input_image).reshape(H, W, C).astype(np.float32, copy=False)
    xp = np.zeros((H + 2, WPAD, C), np.float32)
    xp[1 : H + 1, 1 : W + 1] = x
    band = build_t_band()
    in_maps = []
    for core in range(NCORES):
        lo = core * ROWS_PER_CORE
        sh = xp[lo : lo + ROWS_PER_CORE + 2]  # [130, 1026, 128]
        chunked = np.stack(
            [sh[:, o : o + 128, :] for o in CHUNK_OFF], axis=2
        )  # [130, 128, 9, 128]
        in_maps.append({"x": np.ascontiguousarray(chunked), "band": band})
    return in_maps


def unshard_output(results):
    """results[i]['out'] [nblk, 128, NJ*RBLK] -> [H, W] f32."""
    out = np.empty((H, W), np.float32)
    for core in range(NCORES):
        st = np.asarray(results[core]["out"])  # [nblk, 128, NJ*RBLK]
        nblk = st.shape[0]
        st = st.reshape(nblk, 128, NJ, RBLK)  # [blk, p, j, rr]
        sim = st.transpose(0, 3, 1, 2).reshape(nblk * RBLK, 128, NJ)  # [h, p, j]
        o = out[core * ROWS_PER_CORE : (core + 1) * ROWS_PER_CORE]
        for j in range(8):
            o[:, 126 * j : 126 * j + 126] = sim[:ROWS_PER_CORE, 1:127, j]
        o[:, 1008:1024] = sim[:ROWS_PER_CORE, 111:127, 8]
    return out


_NC_CACHE = {}


def get_nc():
    if "nc" not in _NC_CACHE:
        _NC_CACHE["nc"] = build_bass()
    return _NC_CACHE["nc"]


def kernel(input_image):
    nc = get_nc()
    in_maps = shard_inputs(input_image)
    res = run_bass_kernel_spmd(nc, in_maps, list(range(NCORES)))
    return unshard_output(res.results)


if __name__ == "__main__":
    rng = np.random.default_rng(0)
    x = rng.standard_normal((H, W, 1, C), dtype=np.float32)
    out = kernel(x)
    print(out.shape, out.dtype, out[:2, :4])


# revision 16
# speedup vs baseline: 1.3871x; 1.0164x over previous
"""3x3 neighborhood cosine-similarity sum (minus self) on 8 TRN2 NeuronCores.

Input:  input_image [1024, 1024, 1, 128] float32  (H, W, 1, C)
Output: sim [1024, 1024] float32

Algorithm per pixel: sim = <xn, BoxSum3x3(xn)> - 1, xn = x / max(||x||, eps).

Sharding: H rows split 128/core across 8 cores, 1-row halo (zeros at edges).

V3 design ("overlap chunks + fused vertical"):
 - w split into 9 OVERLAPPING chunks of 128 (stride 126): chunk j holds
   padded w-range [off_j, off_j+128), off = [0,126,...,882, 898] over a
   zero-padded width of 1026. Valid outputs at partitions p in [1,126].
   This makes the horizontal box sum a single in-chunk tridiagonal band
   matmul (no cross-chunk fixup matmuls).
 - vertical 3-row sum fused into PE: S_r = sum_{h=r-1..r+1} T @ xn_h
   accumulated in PSUM across three row iterations (start at h=r-1,
   stop at h=r+1). No DVE vertical adds at all.
 - ss (sum of squares) per chunk via tensor_scalar pow+accum on DVE,
   with some chunks offloaded to ACT (activation Square + accum_out).
 - xn = x * rsqrt(ss+eps) per chunk: split DVE tensor_scalar / ACT copy
   with scale AP.
 - evac: one ACT activation Copy per output row PSUM->SBUF bf16.
 - dot: 9x scalar_tensor_tensor bf16 with accum into staging columns.

Host: pre-chunks the shard to [130, 128, 9, 128] f32 so each partition's
DMA line is 4608B contiguous; output staged [blk, p, j*16+rr], untangled
on host.
"""

import numpy as np
import ml_dtypes

import sys

for _p in ("/opt/trn_rl_repo",):
    if _p not in sys.path:
        sys.path.insert(0, _p)

import concourse.bass as bass
import concourse.bacc as bacc
import concourse.mybir as mybir
import concourse.tile as tile
from concourse.bass_utils import run_bass_kernel_spmd

F32 = mybir.dt.float32
BF16 = mybir.dt.bfloat16
ALU = mybir.AluOpType
ACTF = mybir.ActivationFunctionType

H, W, C = 1024, 1024, 128
NCORES = 8
ROWS_PER_CORE = H // NCORES          # 128
NJ = 9                               # overlapping w-chunks
WPAD = W + 2                         # zero col at 0 and 1025
CHUNK_OFF = [126 * j for j in range(8)] + [WPAD - 128]  # [0..882, 898]
RBLK = 16                            # output rows per staging block

# engine split for xn chunks (tuned from trace):
XN_ACT = (4,)                  # chunk js whose xn runs on ACT


def build_t_band():
    t = np.zeros((128, 128), np.float32)
    for k in range(128):
        for m in (k - 1, k, k + 1):
            if 0 <= m < 128:
                t[k, m] = 1.0
    return t.astype(ml_dtypes.bfloat16)


def build_bass(n_out_rows=ROWS_PER_CORE):
    n_in = n_out_rows + 2
    nblk = (n_out_rows + RBLK - 1) // RBLK

    nc = bacc.Bacc(None, target_bir_lowering=False)
    x_dram = nc.declare_dram_parameter("x", [n_in, 128, NJ, C], F32, isOutput=False)
    band_dram = nc.declare_dram_parameter("band", [128, 128], BF16, isOutput=False)
    out_dram = nc.declare_dram_parameter(
        "out", [nblk, 128, NJ * RBLK], F32, isOutput=True
    )

    with tile.TileContext(nc) as tc:
        with (
            tc.tile_pool(name="consts", bufs=1) as cpool,
            tc.tile_pool(name="xin", bufs=4) as xpool,
            tc.tile_pool(name="sq", bufs=3) as sqpool,
            tc.tile_pool(name="pd", bufs=2) as dpool,
            tc.tile_pool(name="norm", bufs=4) as npool,
            tc.tile_pool(name="xn", bufs=6) as xnpool,
            tc.tile_pool(name="sb", bufs=3) as sbpool,
            tc.tile_pool(name="sim", bufs=2) as simpool,
            tc.tile_pool(name="psum", bufs=1, space="PSUM") as psumpool,
        ):
            band = cpool.tile([128, 128], BF16, tag="band")
            nc.sync.dma_start(band[:], band_dram[:])
            eps_bias = cpool.tile([128, 1], F32, tag="eps")
            nc.gpsimd.memset(eps_bias[:], 1e-16)

            # PSUM: accumulation groups claim whole 2KB banks, so slots must be
            # bank-exclusive. A-slots (chunks 0..7) = 2 banks x3 rotating;
            # B-slots (chunk 8) = 1 full bank x2 rotating. 6 + 2 = 8 banks.
            psum_all = psumpool.tile([128, 32, C], F32, tag="S3")

            xn_rows = [None] * n_in
            simt = None

            for h in range(n_in):
                # ---- load row h: [128 p, 9 j, 128 c]
                xt = xpool.tile([128, NJ, C], F32, tag="xt")
                nc.sync.dma_start(xt[:], x_dram[h])

                # ---- ss[p, j] = sum_c x^2: full-row Square on ACT, then one
                # segmented tensor_reduce on DVE (no accumulator-read tax).
                sq = sqpool.tile([128, NJ, C], BF16, tag="sq")
                nc.scalar.activation(sq[:], xt[:], ACTF.Square)
                ssr = npool.tile([128, NJ], F32, tag="ssr")
                nc.vector.tensor_reduce(
                    ssr[:], sq[:], mybir.AxisListType.X, ALU.add
                )

                # ---- inv = 1 / sqrt(ss + 1e-16)
                snorm = npool.tile([128, NJ], F32, tag="snorm")
                nc.scalar.activation(snorm[:], ssr[:], ACTF.Sqrt, bias=eps_bias[:])
                sinv = npool.tile([128, NJ], F32, tag="sinv")
                nc.vector.reciprocal(sinv[:], snorm[:])

                # ---- xn = x * inv  (f32 -> bf16), per chunk (DVE / ACT split)
                xnt = xnpool.tile([128, NJ, C], BF16, tag="xnt")
                for j in range(NJ):
                    if j in XN_ACT:
                        nc.scalar.activation(
                            xnt[:, j, :], xt[:, j, :], ACTF.Copy,
                            scale=sinv[:, j : j + 1],
                        )
                    else:
                        nc.vector.tensor_scalar(
                            xnt[:, j, :], xt[:, j, :], sinv[:, j : j + 1],
                            None, ALU.mult,
                        )
                xn_rows[h] = xnt

                # ---- PE: accumulate T @ xn_h into S_{h-1}, S_h, S_{h+1}.
                # Emission order matters: stop-pass for S_{h-1}, then its
                # evacs (so the B-bank WAR orders start-after-evac), then
                # mid/start passes.
                def band_pass(r, start, stop):
                    ka_ = (r % 3) * 8          # A-slot base chunk (2 banks)
                    kb_ = 24 + (r % 2) * 4     # B-slot base chunk (1 bank)
                    nc.tensor.matmul(
                        psum_all[:, ka_ : ka_ + 4, :], band[:],
                        xnt[:, 0:4, :], start=start, stop=stop,
                    )
                    nc.tensor.matmul(
                        psum_all[:, ka_ + 4 : ka_ + 8, :], band[:],
                        xnt[:, 4:8, :], start=start, stop=stop,
                    )
                    nc.tensor.matmul(
                        psum_all[:, kb_ : kb_ + 1, :], band[:],
                        xnt[:, 8:9, :], start=start, stop=stop,
                    )

                if h >= 2:
                    band_pass(h - 1, start=False, stop=True)

                    # ---- row r = h-1 complete: evac (B first: frees its bank)
                    r = h - 1
                    ro = r - 1  # 0..n_out_rows-1
                    ka = (r % 3) * 8
                    kb = 24 + (r % 2) * 4
                    xb_ = xn_rows[r]
                    xn_rows[r - 1] = None
                    sb = sbpool.tile([128, NJ, C], BF16, tag="sbt")
                    nc.scalar.activation(
                        sb[:, 8:9, :], psum_all[:, kb : kb + 1, :], ACTF.Copy
                    )
                    nc.scalar.activation(
                        sb[:, 0:8, :], psum_all[:, ka : ka + 8, :], ACTF.Copy
                    )

                if h <= n_out_rows - 1:
                    band_pass(h + 1, start=True, stop=False)
                if 1 <= h <= n_out_rows:
                    band_pass(h, start=False, stop=False)

                if h < 2:
                    continue

                # ---- dot: full-row product then one segmented reduce into
                # the staging tile column rr (strided output, stride RBLK).
                if ro % RBLK == 0:
                    simt = simpool.tile([128, NJ, RBLK], F32, tag="simt")
                rr = ro % RBLK
                pd = dpool.tile([128, NJ, C], BF16, tag="pd")
                nc.vector.tensor_mul(pd[:], xb_[:], sb[:])
                nc.vector.tensor_reduce(
                    simt[:, :, rr], pd[:], mybir.AxisListType.X, ALU.add
                )

                if ro % RBLK == RBLK - 1 or ro == n_out_rows - 1:
                    blk = ro // RBLK
                    simo = simpool.tile([128, NJ, RBLK], F32, tag="simo")
                    nc.vector.tensor_scalar(
                        simo[:], simt[:], -1.0, None, ALU.add
                    )
                    nc.sync.dma_start(out_dram[blk], simo[:])

    nc.compile()
    return nc


def shard_inputs(input_image):
    """input_image [H, W, 1, C] f32 -> per-core in_maps (pre-chunked)."""
    x = np.asarray(input_image).reshape(H, W, C).astype(np.float32, copy=False)
    xp = np.zeros((H + 2, WPAD, C), np.float32)
    xp[1 : H + 1, 1 : W + 1] = x
    band = build_t_band()
    in_maps = []
    for core in range(NCORES):
        lo = core * ROWS_PER_CORE
        sh = xp[lo : lo + ROWS_PER_CORE + 2]  # [130, 1026, 128]
        chunked = np.stack(
            [sh[:, o : o + 128, :] for o in CHUNK_OFF], axis=2
        )  # [130, 128, 9, 128]
        in_maps.append({"x": np.ascontiguousarray(chunked), "band": band})
    return in_maps


def unshard_output(results):
    """results[i]['out'] [nblk, 128, NJ*RBLK] -> [H, W] f32."""
    out = np.empty((H, W), np.float32)
    for core in range(NCORES):
        st = np.asarray(results[core]["out"])  # [nblk, 128, NJ*RBLK]
        nblk = st.shape[0]
        st = st.reshape(nblk, 128, NJ, RBLK)  # [blk, p, j, rr]
        sim = st.transpose(0, 3, 1, 2).reshape(nblk * RBLK, 128, NJ)  # [h, p, j]
        o = out[core * ROWS_PER_CORE : (core + 1) * ROWS_PER_CORE]
        for j in range(8):
            o[:, 126 * j : 126 * j + 126] = sim[:ROWS_PER_CORE, 1:127, j]
        o[:, 1008:1024] = sim[:ROWS_PER_CORE, 111:127, 8]
    return out


_NC_CACHE = {}


def get_nc():
    if "nc" not in _NC_CACHE:
        _NC_CACHE["nc"] = build_bass()
    return _NC_CACHE["nc"]


def kernel(input_image):
    nc = get_nc()
    in_maps = shard_inputs(input_image)
    res = run_bass_kernel_spmd(nc, in_maps, list(range(NCORES)))
    return unshard_output(res.results)


if __name__ == "__main__":
    rng = np.random.default_rng(0)
    x = rng.standard_normal((H, W, 1, C), dtype=np.float32)
    out = kernel(x)
    print(out.shape, out.dtype, out[:2, :4])


# revision 18
# speedup vs baseline: 1.4734x; 1.0622x over previous
"""3x3 neighborhood cosine-similarity sum (minus self) on 8 TRN2 NeuronCores.

Input:  input_image [1024, 1024, 1, 128] float32  (H, W, 1, C)
Output: sim [1024, 1024] float32

Algorithm per pixel: sim = <xn, BoxSum3x3(xn)> - 1, xn = x / max(||x||, eps).

Sharding: H rows split 128/core across 8 cores, 1-row halo (zeros at edges).

V3 design ("overlap chunks + fused vertical"):
 - w split into 9 OVERLAPPING chunks of 128 (stride 126): chunk j holds
   padded w-range [off_j, off_j+128), off = [0,126,...,882, 898] over a
   zero-padded width of 1026. Valid outputs at partitions p in [1,126].
   This makes the horizontal box sum a single in-chunk tridiagonal band
   matmul (no cross-chunk fixup matmuls).
 - vertical 3-row sum fused into PE: S_r = sum_{h=r-1..r+1} T @ xn_h
   accumulated in PSUM across three row iterations (start at h=r-1,
   stop at h=r+1). No DVE vertical adds at all.
 - ss (sum of squares) per chunk via tensor_scalar pow+accum on DVE,
   with some chunks offloaded to ACT (activation Square + accum_out).
 - xn = x * rsqrt(ss+eps) per chunk: split DVE tensor_scalar / ACT copy
   with scale AP.
 - evac: one ACT activation Copy per output row PSUM->SBUF bf16.
 - dot: 9x scalar_tensor_tensor bf16 with accum into staging columns.

Host: pre-chunks the shard to [130, 128, 9, 128] f32 so each partition's
DMA line is 4608B contiguous; output staged [blk, p, j*16+rr], untangled
on host.
"""

import numpy as np
import ml_dtypes

import sys

for _p in ("/opt/trn_rl_repo",):
    if _p not in sys.path:
        sys.path.insert(0, _p)

import concourse.bass as bass
import concourse.bacc as bacc
import concourse.mybir as mybir
import concourse.tile as tile
from concourse.bass_utils import run_bass_kernel_spmd

F32 = mybir.dt.float32
BF16 = mybir.dt.bfloat16
ALU = mybir.AluOpType
ACTF = mybir.ActivationFunctionType

H, W, C = 1024, 1024, 128
NCORES = 8
ROWS_PER_CORE = H // NCORES          # 128
NJ = 9                               # overlapping w-chunks
WPAD = W + 2                         # zero col at 0 and 1025
CHUNK_OFF = [126 * j for j in range(8)] + [WPAD - 128]  # [0..882, 898]
RBLK = 16                            # output rows per staging block

# engine split for xn chunks (tuned from trace):
XN_ACT = (4,)                  # chunk js whose xn runs on ACT


def build_t_band():
    t = np.zeros((128, 128), np.float32)
    for k in range(128):
        for m in (k - 1, k, k + 1):
            if 0 <= m < 128:
                t[k, m] = 1.0
    return t.astype(ml_dtypes.bfloat16)


def build_bass(n_out_rows=ROWS_PER_CORE):
    n_in = n_out_rows + 2
    nblk = (n_out_rows + RBLK - 1) // RBLK

    nc = bacc.Bacc(None, target_bir_lowering=False)
    x_dram = nc.declare_dram_parameter("x", [n_in, 128, NJ, C], F32, isOutput=False)
    band_dram = nc.declare_dram_parameter("band", [128, 128], BF16, isOutput=False)
    out_dram = nc.declare_dram_parameter(
        "out", [nblk, 128, NJ * RBLK], F32, isOutput=True
    )

    with tile.TileContext(nc) as tc:
        with (
            tc.tile_pool(name="consts", bufs=1) as cpool,
            tc.tile_pool(name="xin", bufs=4) as xpool,
            tc.tile_pool(name="sq", bufs=3) as sqpool,
            tc.tile_pool(name="pd", bufs=2) as dpool,
            tc.tile_pool(name="norm", bufs=4) as npool,
            tc.tile_pool(name="xn", bufs=6) as xnpool,
            tc.tile_pool(name="sb", bufs=3) as sbpool,
            tc.tile_pool(name="sim", bufs=2) as simpool,
            tc.tile_pool(name="psum", bufs=1, space="PSUM") as psumpool,
        ):
            band = cpool.tile([128, 128], BF16, tag="band")
            nc.sync.dma_start(band[:], band_dram[:])
            eps_bias = cpool.tile([128, 1], F32, tag="eps")
            nc.gpsimd.memset(eps_bias[:], 1e-16)

            # PSUM: accumulation groups claim whole 2KB banks, so slots must be
            # bank-exclusive. A-slots (chunks 0..7) = 2 banks x3 rotating;
            # B-slots (chunk 8) = 1 full bank x2 rotating. 6 + 2 = 8 banks.
            psum_all = psumpool.tile([128, 32, C], F32, tag="S3")

            xn_rows = [None] * n_in
            simt = None

            for h in range(n_in):
                # ---- load row h: [128 p, 9 j, 128 c], f32 DRAM -> bf16 SBUF
                # (SWDGE cast dma; only Square and xn read x, both bf16-ok)
                xt = xpool.tile([128, NJ, C], BF16, tag="xt")
                nc.gpsimd.dma_start(xt[:], x_dram[h])

                # ---- ss[p, j] = sum_c x^2: full-row Square on ACT, then
                # fold c 128->32 with cheap 2x TT adds so the 1x-rate
                # tensor_reduce only streams a quarter of the elements.
                sq = sqpool.tile([128, NJ, C], BF16, tag="sq")
                nc.scalar.activation(sq[:], xt[:], ACTF.Square)
                sf1 = sqpool.tile([128, NJ, C // 2], BF16, tag="sf1")
                nc.vector.tensor_add(sf1[:], sq[:, :, 0:64], sq[:, :, 64:128])
                sf2 = sqpool.tile([128, NJ, C // 4], BF16, tag="sf2")
                nc.vector.tensor_add(sf2[:], sf1[:, :, 0:32], sf1[:, :, 32:64])
                ssr = npool.tile([128, NJ], F32, tag="ssr")
                nc.vector.tensor_reduce(
                    ssr[:], sf2[:], mybir.AxisListType.X, ALU.add
                )

                # ---- inv = 1 / sqrt(ss + 1e-16)
                snorm = npool.tile([128, NJ], F32, tag="snorm")
                nc.scalar.activation(snorm[:], ssr[:], ACTF.Sqrt, bias=eps_bias[:])
                sinv = npool.tile([128, NJ], F32, tag="sinv")
                nc.vector.reciprocal(sinv[:], snorm[:])

                # ---- xn = x * inv  (f32 -> bf16), per chunk (DVE / ACT split)
                xnt = xnpool.tile([128, NJ, C], BF16, tag="xnt")
                for j in range(NJ):
                    if j in XN_ACT:
                        nc.scalar.activation(
                            xnt[:, j, :], xt[:, j, :], ACTF.Copy,
                            scale=sinv[:, j : j + 1],
                        )
                    else:
                        nc.vector.tensor_scalar(
                            xnt[:, j, :], xt[:, j, :], sinv[:, j : j + 1],
                            None, ALU.mult,
                        )
                xn_rows[h] = xnt

                # ---- PE: accumulate T @ xn_h into S_{h-1}, S_h, S_{h+1}.
                # Emission order matters: stop-pass for S_{h-1}, then its
                # evacs (so the B-bank WAR orders start-after-evac), then
                # mid/start passes.
                def band_pass(r, start, stop):
                    ka_ = (r % 3) * 8          # A-slot base chunk (2 banks)
                    kb_ = 24 + (r % 2) * 4     # B-slot base chunk (1 bank)
                    nc.tensor.matmul(
                        psum_all[:, ka_ : ka_ + 4, :], band[:],
                        xnt[:, 0:4, :], start=start, stop=stop,
                    )
                    nc.tensor.matmul(
                        psum_all[:, ka_ + 4 : ka_ + 8, :], band[:],
                        xnt[:, 4:8, :], start=start, stop=stop,
                    )
                    nc.tensor.matmul(
                        psum_all[:, kb_ : kb_ + 1, :], band[:],
                        xnt[:, 8:9, :], start=start, stop=stop,
                    )

                if h >= 2:
                    band_pass(h - 1, start=False, stop=True)

                    # ---- row r = h-1 complete: evac (B first: frees its bank)
                    r = h - 1
                    ro = r - 1  # 0..n_out_rows-1
                    ka = (r % 3) * 8
                    kb = 24 + (r % 2) * 4
                    xb_ = xn_rows[r]
                    xn_rows[r - 1] = None
                    sb = sbpool.tile([128, NJ, C], BF16, tag="sbt")
                    nc.scalar.activation(
                        sb[:, 8:9, :], psum_all[:, kb : kb + 1, :], ACTF.Copy
                    )
                    nc.scalar.activation(
                        sb[:, 0:8, :], psum_all[:, ka : ka + 8, :], ACTF.Copy
                    )

                if h <= n_out_rows - 1:
                    band_pass(h + 1, start=True, stop=False)
                if 1 <= h <= n_out_rows:
                    band_pass(h, start=False, stop=False)

                if h < 2:
                    continue

                # ---- dot: full-row product then one segmented reduce into
                # the staging tile column rr (strided output, stride RBLK).
                if ro % RBLK == 0:
                    simt = simpool.tile([128, NJ, RBLK], F32, tag="simt")
                rr = ro % RBLK
                pd = dpool.tile([128, NJ, C], BF16, tag="pd")
                nc.vector.tensor_mul(pd[:], xb_[:], sb[:])
                pf1 = dpool.tile([128, NJ, C // 2], BF16, tag="pf1")
                nc.vector.tensor_add(pf1[:], pd[:, :, 0:64], pd[:, :, 64:128])
                pf2 = dpool.tile([128, NJ, C // 4], BF16, tag="pf2")
                nc.vector.tensor_add(pf2[:], pf1[:, :, 0:32], pf1[:, :, 32:64])
                nc.vector.tensor_reduce(
                    simt[:, :, rr], pf2[:], mybir.AxisListType.X, ALU.add
                )

                if ro % RBLK == RBLK - 1 or ro == n_out_rows - 1:
                    blk = ro // RBLK
                    simo = simpool.tile([128, NJ, RBLK], F32, tag="simo")
                    nc.vector.tensor_scalar(
                        simo[:], simt[:], -1.0, None, ALU.add
                    )
                    nc.sync.dma_start(out_dram[blk], simo[:])

    nc.compile()
    return nc


def shard_inputs(input_image):
    """input_image [H, W, 1, C] f32 -> per-core in_maps (pre-chunked)."""
    x = np.asarray(input_image).reshape(H, W, C).astype(np.float32, copy=False)
    xp = np.zeros((H + 2, WPAD, C), np.float32)
    xp[1 : H + 1, 1 : W + 1] = x
    band = build_t_band()
    in_maps = []
    for core in range(NCORES):
        lo = core * ROWS_PER_CORE
        sh = xp[lo : lo + ROWS_PER_CORE + 2]  # [130, 1026, 128]
        chunked = np.stack(
            [sh[:, o : o + 128, :] for o in CHUNK_OFF], axis=2
        )  # [130, 128, 9, 128]
        in_maps.append({"x": np.ascontiguousarray(chunked), "band": band})
    return in_maps


def unshard_output(results):
    """results[i]['out'] [nblk, 128, NJ*RBLK] -> [H, W] f32."""
    out = np.empty((H, W), np.float32)
    for core in range(NCORES):
        st = np.asarray(results[core]["out"])  # [nblk, 128, NJ*RBLK]
        nblk = st.shape[0]
        st = st.reshape(nblk, 128, NJ, RBLK)  # [blk, p, j, rr]
        sim = st.transpose(0, 3, 1, 2).reshape(nblk * RBLK, 128, NJ)  # [h, p, j]
        o = out[core * ROWS_PER_CORE : (core + 1) * ROWS_PER_CORE]
        for j in range(8):
            o[:, 126 * j : 126 * j + 126] = sim[:ROWS_PER_CORE, 1:127, j]
        o[:, 1008:1024] = sim[:ROWS_PER_CORE, 111:127, 8]
    return out


_NC_CACHE = {}


def get_nc():
    if "nc" not in _NC_CACHE:
        _NC_CACHE["nc"] = build_bass()
    return _NC_CACHE["nc"]


def kernel(input_image):
    nc = get_nc()
    in_maps = shard_inputs(input_image)
    res = run_bass_kernel_spmd(nc, in_maps, list(range(NCORES)))
    return unshard_output(res.results)


if __name__ == "__main__":
    rng = np.random.default_rng(0)
    x = rng.standard_normal((H, W, 1, C), dtype=np.float32)
    out = kernel(x)
    print(out.shape, out.dtype, out[:2, :4])
